# revision 19
# baseline (speedup 1.0000x reference)
"""Trainium2 kernel for nn_DecodePredictions (YOLO-style decode + greedy NMS).

Strategy:
  - The memory-bound bulk (reading the [1,512,512,100] f32 input, per-cell
    class-max over 90 classes, per-anchor score = cls_max * objectness) runs
    on 8 NeuronCores, sharded over the 512 grid rows (64 rows per core).
  - Each core streams its 13.1 MB slice through SBUF with big contiguous
    DMAs and produces a [2, 32768] f32 score plane (anchor-major).
  - The greedy NMS only ever touches the top ~30 candidates (the reference
    loop locks onto a zero-area box and repeats it), so it runs on host in
    float32 numpy, replicating the reference op-for-op (verified bitwise
    identical against the JAX reference).
"""

import numpy as np

G = 512
C = 100
NCORES = 8
ROWS = G // NCORES            # 64 grid rows per core
CELLS = ROWS * G              # 32768 cells per core
K = 32                        # cells per SBUF partition per tile
NT = CELLS // (128 * K)       # 8 tiles per core
N_CLS = 90

STRIDE = np.float32(16.0)
INPUT_SIZE = np.float32(8192.0)
IOU_THR = np.float32(0.5)
SCORE_THR = np.float32(0.6)
MAX_OUT = 100

_CACHE = {}


def _legalize_multi_waits(nc):
    """This toolchain's walrus rejects any instruction whose sync_info
    carries 2+ waits ("Too many sync wait commands" — the TPB EVENTS slot
    holds exactly one wait). Tile emits multi-wait instructions expecting
    the backend to split them, so do the split here: hoist all but the
    last wait onto single-wait engine NoOps inserted just before the
    instruction (same engine stream => same ordering guarantee)."""
    import concourse.mybir as mybir

    ctr = 0
    for bb in nc.main_func.blocks:
        out = []
        for ins in bb.instructions:
            si = getattr(ins, "sync_info", None)
            waits = list(si.on_wait) if (si is not None and si.on_wait) else []
            if len(waits) > 1:
                for w in waits[:-1]:
                    ctr += 1
                    nop = mybir.InstNoOp(
                        name=f"I-waitsplit-{ctr}", engine=ins.engine
                    )
                    nop.sync_info = mybir.SyncInfo(on_wait=[w], on_update=[])
                    out.append(nop)
                ins.sync_info = mybir.SyncInfo(
                    on_wait=[waits[-1]], on_update=list(si.on_update or [])
                )
            out.append(ins)
        if len(out) != len(bb.instructions):
            bb.instructions = out
    return nc


def _build_score_program():
    """Bass/Tile program: x[CELLS, 100] -> s[CELLS, 2] with
    s[cell, a] = max(x[cell, 10:100]) * x[cell, a].

    Cells are processed partition-major: partition p owns cells
    [p*NT*K, (p+1)*NT*K), split into NT tiles of K cells. All scores
    accumulate into one persistent SBUF tile; a single output DMA at the
    end writes [CELLS, 2] (keeps every DMA at <=1 sync wait — walrus
    rejects DMACopy instructions with 2+ waits)."""
    import concourse.bass as bass
    import concourse.mybir as mybir
    from concourse import tile

    nc = bass.Bass()
    x = nc.declare_dram_parameter("x", [CELLS, C], mybir.dt.float32, isOutput=False)
    s = nc.declare_dram_parameter("s", [CELLS, 2], mybir.dt.float32, isOutput=True)

    # cell = p*(NT*K) + t*K + k
    xv = x.rearrange("(p t k) c -> t p (k c)", p=128, t=NT)     # [NT, 128, K*C]
    sv3 = s.rearrange("(p t ka) a -> t p (ka a)", p=128, t=NT)  # [NT, 128, K*2]

    with tile.TileContext(nc) as tc:
        with (
            tc.tile_pool(name="xin", bufs=NT) as pin,
            tc.tile_pool(name="mx", bufs=2) as pmx,
            tc.tile_pool(name="out", bufs=1) as pout,
        ):
            so = pout.tile([128, NT * K * 2], mybir.dt.float32)
            s4 = so[:].rearrange("p (t k a) -> p t k a", t=NT, a=2)
            for t in range(NT):
                xt = pin.tile([128, K * C], mybir.dt.float32)
                nc.sync.dma_start(xt[:], xv[t])
                x3 = xt[:].rearrange("p (k c) -> p k c", c=C)
                mx = pmx.tile([128, K], mybir.dt.float32)
                nc.vector.reduce_max(
                    out=mx[:], in_=x3[:, :, 10:C], axis=mybir.AxisListType.X
                )
                mxb = mx[:].rearrange("p (k o) -> p k o", o=1).broadcast_to(
                    [128, K, 2]
                )
                nc.vector.tensor_mul(
                    out=s4[:, t, :, :], in0=mxb, in1=x3[:, :, 0:2]
                )
                # per-tile store on the otherwise-idle ACT HWDGE ring, so
                # score stores overlap the input stream without stalling it
                nc.scalar.dma_start(
                    sv3[t], so[:, t * K * 2 : (t + 1) * K * 2]
                )
    return _legalize_multi_waits(nc)


def _strip_preamble_barrier(nc):
    """Remove the all-engine barrier bass emits after its const-tile
    memsets (block 0). Our program doesn't read the const tiles, and
    every real ordering need is covered by per-engine program order
    (each engine's register moves precede its own work) and data
    semaphores. The barrier round is self-resetting (gather/release end
    back at 0), so dropping the whole round leaves the tail 'aeb'
    barrier round unaffected. Saves ~3us of kernel head."""
    import concourse.mybir as mybir

    bb = nc.main_func.blocks[0]
    keep = [
        ins
        for ins in bb.instructions
        if not (
            isinstance(ins, mybir.InstDrain)
            or (
                isinstance(ins, mybir.InstEventSemaphore)
                and str(ins.name).startswith("barrier_")
            )
        )
    ]
    if len(keep) != len(bb.instructions):
        bb.instructions = keep
    return nc


def _build_score_program_raw():
    """Raw-Bass (no Tile) variant: identical dataflow to
    _build_score_program but with hand-rolled semaphores and no Tile
    prologue/epilogue barriers (saves ~15us of fixed overhead).

    SP issues the NT input chains back-to-back on its HWDGE ring; DVE
    consumes tiles in ring order (reduce_max + broadcast mul); ACT
    issues the per-tile score stores on its own HWDGE ring and finally
    waits for all stores to land."""
    import contextlib

    import concourse.bass as bass
    import concourse.mybir as mybir

    nc = bass.Bass()
    x = nc.declare_dram_parameter("x", [CELLS, C], mybir.dt.float32, isOutput=False)
    s = nc.declare_dram_parameter("s", [CELLS, 2], mybir.dt.float32, isOutput=True)

    xv = x.rearrange("(p t k) c -> t p (k c)", p=128, t=NT)     # [NT, 128, K*C]
    sv3 = s.rearrange("(p t ka) a -> t p (ka a)", p=128, t=NT)  # [NT, 128, K*2]

    with contextlib.ExitStack() as ctx:
        xt = [
            ctx.enter_context(
                nc.sbuf_tensor(f"xt{t}", [128, K * C], mybir.dt.float32)
            )
            for t in range(NT)
        ]
        mx = [
            ctx.enter_context(
                nc.sbuf_tensor(f"mx{t}", [128, K], mybir.dt.float32)
            )
            for t in range(NT)
        ]
        so = ctx.enter_context(
            nc.sbuf_tensor("so", [128, NT * K * 2], mybir.dt.float32)
        )
        # One completion semaphore per input chain: a chain's 16 DMA-engine
        # parts each inc +1, and engines process chains at different speeds,
        # so a single shared counter at 16*(t+1) could mix parts of later
        # chains while one engine still owes tile t data (observed as
        # intermittent first-run corruption).
        in_sems = [
            ctx.enter_context(nc.semaphore(f"in_sem{t}")) for t in range(NT)
        ]
        dve_sem = ctx.enter_context(nc.semaphore("dve_sem"))
        out_sem = ctx.enter_context(nc.semaphore("out_sem"))
        block = ctx.enter_context(nc.Block(no_gpsimd_drain=True))

        s4 = so[:].rearrange("p (t k a) -> p t k a", t=NT, a=2)

        @block.sync
        def _(sync):
            for t in range(NT):
                sync.dma_start(xt[t][:], xv[t]).then_inc(in_sems[t], 16)

        @block.vector
        def _(vector):
            for t in range(NT):
                vector.wait_ge(in_sems[t], 16)
                x3 = xt[t][:].rearrange("p (k c) -> p k c", c=C)
                nc.vector.reduce_max(
                    out=mx[t][:], in_=x3[:, :, 10:C], axis=mybir.AxisListType.X
                )
                mxb = mx[t][:].rearrange("p (k o) -> p k o", o=1).broadcast_to(
                    [128, K, 2]
                )
                nc.vector.tensor_mul(
                    out=s4[:, t, :, :], in0=mxb, in1=x3[:, :, 0:2]
                ).then_inc(dve_sem, 1)

        @block.scalar
        def _(scalar):
            for t in range(NT):
                scalar.wait_ge(dve_sem, t + 1)
                scalar.dma_start(
                    sv3[t], so[:, t * K * 2 : (t + 1) * K * 2]
                ).then_inc(out_sem, 16)
            scalar.wait_ge(out_sem, 16 * NT)

    return nc


def _get_program():
    if "nc" not in _CACHE:
        _CACHE["nc"] = _build_score_program_raw()
    return _CACHE["nc"]


def device_scores(xf, trace=False):
    """Run the 8-core score kernel. xf: [G*G, C] f32 contiguous.
    Returns probs [G*G*2] f32 in reference anchor order (n = cell*2 + a),
    plus the BassKernelResults (for profiling when trace=True)."""
    from concourse.bass_utils import run_bass_kernel_spmd

    nc = _get_program()
    in_maps = [
        {"x": xf[i * CELLS : (i + 1) * CELLS]} for i in range(NCORES)
    ]
    res = run_bass_kernel_spmd(
        nc, in_maps, list(range(NCORES)), trace=trace
    )
    # r["s"] is [CELLS, 2] cell-major, so per-core flattening is already
    # the reference anchor order (n_local = cell_local*2 + a).
    probs = np.concatenate(
        [r["s"].reshape(-1) for r in res.results]
    ).astype(np.float32, copy=False)
    return probs, res


def _decode_boxes_for(xf, n):
    """Exact fp32 decode of boxes_xyxy for global anchor indices n."""
    cell = n >> 1
    a = (n & 1).astype(np.int64)
    colf = (cell % G).astype(np.float32)
    rowf = (cell // G).astype(np.float32)
    base = 2 + 4 * a
    cx = (xf[cell, base + 0] + colf) * STRIDE
    cy = (xf[cell, base + 1] + rowf) * STRIDE
    w = np.square(xf[cell, base + 2]) * INPUT_SIZE
    h = np.square(xf[cell, base + 3]) * INPUT_SIZE
    half_w = w / np.float32(2.0)
    half_h = h / np.float32(2.0)
    x1 = cx - half_w
    y1 = cy - half_h
    x2 = cx + half_w - np.float32(1.0)
    y2 = cy + half_h - np.float32(1.0)
    return x1, y1, x2, y2


def _decode_dense(xf, square_wh):
    """Full dense decode (only used when extract_boxes is falsy)."""
    cell = np.arange(G * G)
    colf = (cell % G).astype(np.float32)[:, None]
    rowf = (cell // G).astype(np.float32)[:, None]
    bb = xf[:, 2:10].reshape(G * G, 2, 4)
    cx = (bb[:, :, 0] + colf) * STRIDE
    cy = (bb[:, :, 1] + rowf) * STRIDE
    wh = bb[:, :, 2:4]
    if square_wh:
        wh = np.square(wh)
    wh = wh * INPUT_SIZE
    out = np.concatenate(
        [cx[..., None], cy[..., None], wh], axis=-1
    ).astype(np.float32)
    return out.reshape(1, G, G, 2, 4)


def kernel(inputs, square_wh, extract_boxes, _trace=False):
    x = np.ascontiguousarray(np.asarray(inputs), dtype=np.float32)
    xf = x.reshape(G * G, C)

    if not extract_boxes:
        return _decode_dense(xf, square_wh)

    probs, res = device_scores(xf, trace=_trace)
    sel = _greedy_nms_generic(xf, probs, square_wh)

    valid = sel >= 0
    idx = np.maximum(sel, 0)
    x1, y1, x2, y2 = _decode_boxes_generic(xf, idx, square_wh)
    boxes = np.stack([x1, y1, x2, y2], axis=1).astype(np.float32)
    nms_boxes = np.where(valid[:, None], boxes, np.float32(0.0)).astype(np.float32)
    nms_scores = np.where(valid, probs[idx], np.float32(0.0)).astype(np.float32)
    sel_cells = (idx >> 1)
    cls_sel = np.argmax(xf[sel_cells, 10:C], axis=1).astype(np.int32)
    nms_cls_ids = np.where(valid, cls_sel, np.int32(-1)).astype(np.int32)
    if _trace:
        return (nms_boxes, nms_cls_ids, nms_scores, valid), res
    return nms_boxes, nms_cls_ids, nms_scores, valid


def _decode_boxes_generic(xf, n, square_wh):
    if square_wh:
        return _decode_boxes_for(xf, n)
    cell = n >> 1
    a = (n & 1).astype(np.int64)
    colf = (cell % G).astype(np.float32)
    rowf = (cell // G).astype(np.float32)
    base = 2 + 4 * a
    cx = (xf[cell, base + 0] + colf) * STRIDE
    cy = (xf[cell, base + 1] + rowf) * STRIDE
    w = xf[cell, base + 2] * INPUT_SIZE
    h = xf[cell, base + 3] * INPUT_SIZE
    x1 = cx - w / np.float32(2.0)
    y1 = cy - h / np.float32(2.0)
    x2 = cx + w / np.float32(2.0) - np.float32(1.0)
    y2 = cy + h / np.float32(2.0) - np.float32(1.0)
    return x1, y1, x2, y2


def _greedy_nms_generic(xf, probs, square_wh):
    cand = np.nonzero(probs > SCORE_THR)[0]
    sel = np.full(MAX_OUT, -1, np.int64)
    if cand.size == 0:
        return sel
    cs = probs[cand]
    x1, y1, x2, y2 = _decode_boxes_generic(xf, cand, square_wh)
    area = np.maximum(x2 - x1, np.float32(0.0)) * np.maximum(
        y2 - y1, np.float32(0.0)
    )
    active = np.ones(cand.size, bool)
    neg = np.float32(-1e30)
    for k in range(MAX_OUT):
        masked = np.where(active, cs, neg)
        i = int(np.argmax(masked))
        if not (masked[i] > neg):
            break
        ix1 = np.maximum(x1, x1[i])
        iy1 = np.maximum(y1, y1[i])
        ix2 = np.minimum(x2, x2[i])
        iy2 = np.minimum(y2, y2[i])
        inter = np.maximum(ix2 - ix1, np.float32(0.0)) * np.maximum(
            iy2 - iy1, np.float32(0.0)
        )
        iou = inter / (area + area[i] - inter + np.float32(1e-9))
        active = active & (iou <= IOU_THR)
        sel[k] = cand[i]
    return sel


# revision 20
# speedup vs baseline: 1.0467x; 1.0467x over previous
"""Trainium2 kernel for nn_DecodePredictions (YOLO-style decode + greedy NMS).

Strategy:
  - The memory-bound bulk (reading the [1,512,512,100] f32 input, per-cell
    class-max over 90 classes, per-anchor score = cls_max * objectness) runs
    on 8 NeuronCores, sharded over the 512 grid rows (64 rows per core).
  - Each core streams its 13.1 MB slice through SBUF with big contiguous
    DMAs and produces a [2, 32768] f32 score plane (anchor-major).
  - The greedy NMS only ever touches the top ~30 candidates (the reference
    loop locks onto a zero-area box and repeats it), so it runs on host in
    float32 numpy, replicating the reference op-for-op (verified bitwise
    identical against the JAX reference).
"""

import numpy as np

G = 512
C = 100
NCORES = 8
ROWS = G // NCORES            # 64 grid rows per core
CELLS = ROWS * G              # 32768 cells per core
K = 32                        # cells per SBUF partition per tile
NT = CELLS // (128 * K)       # 8 tiles per core
N_CLS = 90

STRIDE = np.float32(16.0)
INPUT_SIZE = np.float32(8192.0)
IOU_THR = np.float32(0.5)
SCORE_THR = np.float32(0.6)
MAX_OUT = 100

_CACHE = {}


def _legalize_multi_waits(nc):
    """This toolchain's walrus rejects any instruction whose sync_info
    carries 2+ waits ("Too many sync wait commands" — the TPB EVENTS slot
    holds exactly one wait). Tile emits multi-wait instructions expecting
    the backend to split them, so do the split here: hoist all but the
    last wait onto single-wait engine NoOps inserted just before the
    instruction (same engine stream => same ordering guarantee)."""
    import concourse.mybir as mybir

    ctr = 0
    for bb in nc.main_func.blocks:
        out = []
        for ins in bb.instructions:
            si = getattr(ins, "sync_info", None)
            waits = list(si.on_wait) if (si is not None and si.on_wait) else []
            if len(waits) > 1:
                for w in waits[:-1]:
                    ctr += 1
                    nop = mybir.InstNoOp(
                        name=f"I-waitsplit-{ctr}", engine=ins.engine
                    )
                    nop.sync_info = mybir.SyncInfo(on_wait=[w], on_update=[])
                    out.append(nop)
                ins.sync_info = mybir.SyncInfo(
                    on_wait=[waits[-1]], on_update=list(si.on_update or [])
                )
            out.append(ins)
        if len(out) != len(bb.instructions):
            bb.instructions = out
    return nc


def _build_score_program():
    """Bass/Tile program: x[CELLS, 100] -> s[CELLS, 2] with
    s[cell, a] = max(x[cell, 10:100]) * x[cell, a].

    Cells are processed partition-major: partition p owns cells
    [p*NT*K, (p+1)*NT*K), split into NT tiles of K cells. All scores
    accumulate into one persistent SBUF tile; a single output DMA at the
    end writes [CELLS, 2] (keeps every DMA at <=1 sync wait — walrus
    rejects DMACopy instructions with 2+ waits)."""
    import concourse.bass as bass
    import concourse.mybir as mybir
    from concourse import tile

    nc = bass.Bass()
    x = nc.declare_dram_parameter("x", [CELLS, C], mybir.dt.float32, isOutput=False)
    s = nc.declare_dram_parameter("s", [CELLS, 2], mybir.dt.float32, isOutput=True)

    # cell = p*(NT*K) + t*K + k
    xv = x.rearrange("(p t k) c -> t p (k c)", p=128, t=NT)     # [NT, 128, K*C]
    sv3 = s.rearrange("(p t ka) a -> t p (ka a)", p=128, t=NT)  # [NT, 128, K*2]

    with tile.TileContext(nc) as tc:
        with (
            tc.tile_pool(name="xin", bufs=NT) as pin,
            tc.tile_pool(name="mx", bufs=2) as pmx,
            tc.tile_pool(name="out", bufs=1) as pout,
        ):
            so = pout.tile([128, NT * K * 2], mybir.dt.float32)
            s4 = so[:].rearrange("p (t k a) -> p t k a", t=NT, a=2)
            for t in range(NT):
                xt = pin.tile([128, K * C], mybir.dt.float32)
                nc.sync.dma_start(xt[:], xv[t])
                x3 = xt[:].rearrange("p (k c) -> p k c", c=C)
                mx = pmx.tile([128, K], mybir.dt.float32)
                nc.vector.reduce_max(
                    out=mx[:], in_=x3[:, :, 10:C], axis=mybir.AxisListType.X
                )
                mxb = mx[:].rearrange("p (k o) -> p k o", o=1).broadcast_to(
                    [128, K, 2]
                )
                nc.vector.tensor_mul(
                    out=s4[:, t, :, :], in0=mxb, in1=x3[:, :, 0:2]
                )
                # per-tile store on the otherwise-idle ACT HWDGE ring, so
                # score stores overlap the input stream without stalling it
                nc.scalar.dma_start(
                    sv3[t], so[:, t * K * 2 : (t + 1) * K * 2]
                )
    return _legalize_multi_waits(nc)


def _strip_preamble_barrier(nc):
    """Remove the all-engine barrier bass emits after its const-tile
    memsets (block 0). Our program doesn't read the const tiles, and
    every real ordering need is covered by per-engine program order
    (each engine's register moves precede its own work) and data
    semaphores. The barrier round is self-resetting (gather/release end
    back at 0), so dropping the whole round leaves the tail 'aeb'
    barrier round unaffected. Saves ~3us of kernel head."""
    import concourse.mybir as mybir

    bb = nc.main_func.blocks[0]
    keep = [
        ins
        for ins in bb.instructions
        if not (
            isinstance(ins, mybir.InstDrain)
            or (
                isinstance(ins, mybir.InstEventSemaphore)
                and str(ins.name).startswith("barrier_")
            )
        )
    ]
    if len(keep) != len(bb.instructions):
        bb.instructions = keep
    return nc


def _build_score_program_raw():
    """Raw-Bass (no Tile) variant: identical dataflow to
    _build_score_program but with hand-rolled semaphores and no Tile
    prologue/epilogue barriers (saves ~15us of fixed overhead).

    SP issues the NT input chains back-to-back on its HWDGE ring; DVE
    consumes tiles in ring order (reduce_max + broadcast mul); ACT
    issues the per-tile score stores on its own HWDGE ring and finally
    waits for all stores to land."""
    import contextlib

    import concourse.bass as bass
    import concourse.mybir as mybir

    nc = bass.Bass()
    x = nc.declare_dram_parameter("x", [CELLS, C], mybir.dt.float32, isOutput=False)
    s = nc.declare_dram_parameter("s", [CELLS, 2], mybir.dt.float32, isOutput=True)

    xv = x.rearrange("(p t k) c -> t p (k c)", p=128, t=NT)     # [NT, 128, K*C]
    sv3 = s.rearrange("(p t ka) a -> t p (ka a)", p=128, t=NT)  # [NT, 128, K*2]

    with contextlib.ExitStack() as ctx:
        xt = [
            ctx.enter_context(
                nc.sbuf_tensor(f"xt{t}", [128, K * C], mybir.dt.float32)
            )
            for t in range(NT)
        ]
        mx = [
            ctx.enter_context(
                nc.sbuf_tensor(f"mx{t}", [128, K], mybir.dt.float32)
            )
            for t in range(NT)
        ]
        so = ctx.enter_context(
            nc.sbuf_tensor("so", [128, NT * K * 2], mybir.dt.float32)
        )
        # One completion semaphore per input chain: a chain's 16 DMA-engine
        # parts each inc +1, and engines process chains at different speeds,
        # so a single shared counter at 16*(t+1) could mix parts of later
        # chains while one engine still owes tile t data (observed as
        # intermittent first-run corruption).
        in_sems = [
            ctx.enter_context(nc.semaphore(f"in_sem{t}")) for t in range(NT)
        ]
        dve_sem = ctx.enter_context(nc.semaphore("dve_sem"))
        out_sem = ctx.enter_context(nc.semaphore("out_sem"))
        block = ctx.enter_context(nc.Block(no_gpsimd_drain=True))

        s4 = so[:].rearrange("p (t k a) -> p t k a", t=NT, a=2)

        @block.sync
        def _(sync):
            for t in range(NT):
                sync.dma_start(xt[t][:], xv[t]).then_inc(in_sems[t], 16)

        @block.vector
        def _(vector):
            for t in range(NT):
                vector.wait_ge(in_sems[t], 16)
                x3 = xt[t][:].rearrange("p (k c) -> p k c", c=C)
                nc.vector.reduce_max(
                    out=mx[t][:], in_=x3[:, :, 10:C], axis=mybir.AxisListType.X
                )
                mxb = mx[t][:].rearrange("p (k o) -> p k o", o=1).broadcast_to(
                    [128, K, 2]
                )
                nc.vector.tensor_mul(
                    out=s4[:, t, :, :], in0=mxb, in1=x3[:, :, 0:2]
                ).then_inc(dve_sem, 1)

        @block.scalar
        def _(scalar):
            for t in range(NT):
                scalar.wait_ge(dve_sem, t + 1)
                scalar.dma_start(
                    sv3[t], so[:, t * K * 2 : (t + 1) * K * 2]
                ).then_inc(out_sem, 16)
            scalar.wait_ge(out_sem, 16 * NT)

    return _strip_preamble_barrier(nc)


def _get_program():
    if "nc" not in _CACHE:
        _CACHE["nc"] = _build_score_program_raw()
    return _CACHE["nc"]


def device_scores(xf, trace=False):
    """Run the 8-core score kernel. xf: [G*G, C] f32 contiguous.
    Returns probs [G*G*2] f32 in reference anchor order (n = cell*2 + a),
    plus the BassKernelResults (for profiling when trace=True)."""
    from concourse.bass_utils import run_bass_kernel_spmd

    nc = _get_program()
    in_maps = [
        {"x": xf[i * CELLS : (i + 1) * CELLS]} for i in range(NCORES)
    ]
    res = run_bass_kernel_spmd(
        nc, in_maps, list(range(NCORES)), trace=trace
    )
    # r["s"] is [CELLS, 2] cell-major, so per-core flattening is already
    # the reference anchor order (n_local = cell_local*2 + a).
    probs = np.concatenate(
        [r["s"].reshape(-1) for r in res.results]
    ).astype(np.float32, copy=False)
    return probs, res


def _decode_boxes_for(xf, n):
    """Exact fp32 decode of boxes_xyxy for global anchor indices n."""
    cell = n >> 1
    a = (n & 1).astype(np.int64)
    colf = (cell % G).astype(np.float32)
    rowf = (cell // G).astype(np.float32)
    base = 2 + 4 * a
    cx = (xf[cell, base + 0] + colf) * STRIDE
    cy = (xf[cell, base + 1] + rowf) * STRIDE
    w = np.square(xf[cell, base + 2]) * INPUT_SIZE
    h = np.square(xf[cell, base + 3]) * INPUT_SIZE
    half_w = w / np.float32(2.0)
    half_h = h / np.float32(2.0)
    x1 = cx - half_w
    y1 = cy - half_h
    x2 = cx + half_w - np.float32(1.0)
    y2 = cy + half_h - np.float32(1.0)
    return x1, y1, x2, y2


def _decode_dense(xf, square_wh):
    """Full dense decode (only used when extract_boxes is falsy)."""
    cell = np.arange(G * G)
    colf = (cell % G).astype(np.float32)[:, None]
    rowf = (cell // G).astype(np.float32)[:, None]
    bb = xf[:, 2:10].reshape(G * G, 2, 4)
    cx = (bb[:, :, 0] + colf) * STRIDE
    cy = (bb[:, :, 1] + rowf) * STRIDE
    wh = bb[:, :, 2:4]
    if square_wh:
        wh = np.square(wh)
    wh = wh * INPUT_SIZE
    out = np.concatenate(
        [cx[..., None], cy[..., None], wh], axis=-1
    ).astype(np.float32)
    return out.reshape(1, G, G, 2, 4)


def kernel(inputs, square_wh, extract_boxes, _trace=False):
    x = np.ascontiguousarray(np.asarray(inputs), dtype=np.float32)
    xf = x.reshape(G * G, C)

    if not extract_boxes:
        return _decode_dense(xf, square_wh)

    probs, res = device_scores(xf, trace=_trace)
    sel = _greedy_nms_generic(xf, probs, square_wh)

    valid = sel >= 0
    idx = np.maximum(sel, 0)
    x1, y1, x2, y2 = _decode_boxes_generic(xf, idx, square_wh)
    boxes = np.stack([x1, y1, x2, y2], axis=1).astype(np.float32)
    nms_boxes = np.where(valid[:, None], boxes, np.float32(0.0)).astype(np.float32)
    nms_scores = np.where(valid, probs[idx], np.float32(0.0)).astype(np.float32)
    sel_cells = (idx >> 1)
    cls_sel = np.argmax(xf[sel_cells, 10:C], axis=1).astype(np.int32)
    nms_cls_ids = np.where(valid, cls_sel, np.int32(-1)).astype(np.int32)
    if _trace:
        return (nms_boxes, nms_cls_ids, nms_scores, valid), res
    return nms_boxes, nms_cls_ids, nms_scores, valid


def _decode_boxes_generic(xf, n, square_wh):
    if square_wh:
        return _decode_boxes_for(xf, n)
    cell = n >> 1
    a = (n & 1).astype(np.int64)
    colf = (cell % G).astype(np.float32)
    rowf = (cell // G).astype(np.float32)
    base = 2 + 4 * a
    cx = (xf[cell, base + 0] + colf) * STRIDE
    cy = (xf[cell, base + 1] + rowf) * STRIDE
    w = xf[cell, base + 2] * INPUT_SIZE
    h = xf[cell, base + 3] * INPUT_SIZE
    x1 = cx - w / np.float32(2.0)
    y1 = cy - h / np.float32(2.0)
    x2 = cx + w / np.float32(2.0) - np.float32(1.0)
    y2 = cy + h / np.float32(2.0) - np.float32(1.0)
    return x1, y1, x2, y2


def _greedy_nms_generic(xf, probs, square_wh):
    cand = np.nonzero(probs > SCORE_THR)[0]
    sel = np.full(MAX_OUT, -1, np.int64)
    if cand.size == 0:
        return sel
    cs = probs[cand]
    x1, y1, x2, y2 = _decode_boxes_generic(xf, cand, square_wh)
    area = np.maximum(x2 - x1, np.float32(0.0)) * np.maximum(
        y2 - y1, np.float32(0.0)
    )
    active = np.ones(cand.size, bool)
    neg = np.float32(-1e30)
    for k in range(MAX_OUT):
        masked = np.where(active, cs, neg)
        i = int(np.argmax(masked))
        if not (masked[i] > neg):
            break
        ix1 = np.maximum(x1, x1[i])
        iy1 = np.maximum(y1, y1[i])
        ix2 = np.minimum(x2, x2[i])
        iy2 = np.minimum(y2, y2[i])
        inter = np.maximum(ix2 - ix1, np.float32(0.0)) * np.maximum(
            iy2 - iy1, np.float32(0.0)
        )
        iou = inter / (area + area[i] - inter + np.float32(1e-9))
        active = active & (iou <= IOU_THR)
        sel[k] = cand[i]
    return sel
